# revision 58
# baseline (speedup 1.0000x reference)
"""Trainium2 Bass kernel for nn_Cross_AttentionHead_withMask.

Cross-attention head: q = rope(x_text @ Wq.T), k = rope2d(x_image @ Wk.T),
v = x_image @ Wv.T, out = softmax(q k^T / sqrt(512)) v.
(x_latex_mask is accepted but unused - it is dead in the reference.)

Sharding: data-parallel over batch B=8, one batch per NeuronCore (8 cores).

Per-core schedule (all matmuls bf16, accumulation/softmax stats fp32):
  - The exp over the [2048, 4096] score matrix is split between the Act
    engine (exact exp, ~40 tiles) and the Vector engine (~24 tiles via a
    Schraudolph bit-trick: i16 = rint(x*128*log2e + 16249) bitcast to bf16
    gives exp(x) with ~1.8% rms log error; softmax normalization cancels
    most of it - measured end-to-end rel err ~9e-3 vs the 2e-2 gate).
  - K2 packed as [128, TK/2]: rows 0:64 = K.T for the FIRST half of each
    1024-col xt slab, rows 64:128 = the second half (scores pair P couples
    t-tiles (P+4*(P//4), +4)); a scores step computes the pair via two
    row-group-concurrent matmuls, and every k matmul reads a contiguous
    512-col xt range so only half a slab's DMA gates it. k matmuls sit one
    step into each build window (ph1-2) to match slab DMA arrival.
  - head dim is permuted so RoPE partners sit 16 partitions apart within
    each 32-partition quadrant: the partner build is ONE stream_shuffle
    (mask = XOR 16) instead of four 32-row copies.
  - rope muls/adds are split between Vector and GpSimd (gpsimd does the
    SBUF-only tensor_mul/tensor_add legs; it cannot touch PSUM).
  - a warm-up train of tiny matmuls runs during the initial DMA wait so the
    PE HAM clock-gate reaches 2.4 GHz before the first real matmul.
  - attention-out: pso[h, s] += v_aug[t-tile].T @ et, with a ones column in
    v_aug accumulating the softmax denominator for free (v_aug tiles are
    80 wide: 64 head dims + ones + 15 zero pad for the xbar's 16-divisible
    partition rule).
  - epilogue per s-chunk: transpose [80, 512] -> [128, 4, 80] via the DMA
    xbar (PE-transpose only for the tail s-chunk where the PE is idle),
    reciprocal of the Z columns, tensor_scalar multiply, DMA out. Epilogue 0
    sits on step 32 (a DVE-exp step) so its Act osb-copy frees pso0 early
    for att(2,0); q chunks 2/3 sit on steps 25/27 (Act-exp steps) so their
    DVE copies don't stack on DVE-exp steps.
  - CAUTION: performance is extremely sensitive to emission order (Tile sem
    assignment + in-order PE dispatch). Change ONE thing per measurement;
    bundled "obvious" improvements repeatedly hid 15-20us regressions.
"""
import numpy as np
from contextlib import ExitStack

import ml_dtypes

B, TQ, TK = 8, 2048, 4096
DIM_IMG, DIM_TXT, HS = 512, 128, 64
N_CORES = 8
SCALE = float(DIM_IMG) ** -0.5  # reference scales by sqrt(image embed dim)

BF16 = ml_dtypes.bfloat16

# Schraudolph constants for the DVE exp: i16 = x*SCALE*A16 + B16, bitcast bf16
A16 = 128.0 / float(np.log(2.0))
B16 = 16249.0
SHUF_MASK = list(range(16, 32)) + list(range(0, 16))  # XOR-16 within quadrant

_prog_cache = {}


def _patch_tile_drain():
    """This walrus build rejects a Drain carrying >1 sem wait; split the
    TileContext exit waits onto one-wait NoOps."""
    import concourse.tile as tile
    from concourse import mybir
    from concourse.vector_clock import ScopedClock

    if getattr(tile.TileContext, "_drain_patched", False):
        return

    def _drain_and_barrier(self, tick_clock, wait_clock):
        nc = self.nc
        nop = nc.sync.nop()
        wait_clock.add_sem_waits(nop.ins, ScopedClock({None: tick_clock.global_clock}))
        si = nop.ins.sync_info
        waits = list(si.on_wait) if si is not None else []
        if len(waits) > 1:
            nop.ins.sync_info = mybir.SyncInfo(on_wait=[waits[0]], on_update=[])
            for w in waits[1:]:
                extra = nc.sync.nop()
                extra.ins.sync_info = mybir.SyncInfo(on_wait=[w], on_update=[])
        nc.sync.drain()
        nc.all_engine_barrier()
        assert self.sems is not None
        popped = nc._tile_sem_poison_stack.pop()
        assert popped is self._sem_poison
        nc.clear_and_free_semaphores(list(self.sems.allocated().values()))
        nc.all_engine_barrier()

    tile.TileContext._drain_and_barrier = _drain_and_barrier
    tile.TileContext._drain_patched = True


def _split_excess_waits(nc):
    """This walrus build caps sem waits per instruction (1 for DMA/Drain-style
    control instructions, 2 for compute). Move excess waits onto same-engine
    NoOps inserted right before the offending instruction - the engine queue
    is FIFO, so blocking dispatch on the NoOp is semantically equivalent."""
    from concourse import mybir

    ctr = 0
    for fn in nc.m.functions:
        for b in fn.blocks:
            il = b.instructions
            new = []
            changed = False
            for inst in il:
                si = inst.sync_info
                waits = list(si.on_wait) if si is not None else []
                lim = 1
                if len(waits) > lim:
                    for w in waits[lim:]:
                        nop = mybir.InstNoOp(name=f"wsplit-{ctr}", ins=[], outs=[])
                        ctr += 1
                        nop.engine = inst.engine
                        nop.sync_info = mybir.SyncInfo(on_wait=[w], on_update=[])
                        new.append(nop)
                    inst.sync_info = mybir.SyncInfo(
                        on_wait=waits[:lim], on_update=list(si.on_update)
                    )
                    changed = True
                new.append(inst)
            if changed:
                b.instructions = new


def build_program(split_waits=True):
    """Build the single-core Bass program (same program runs SPMD on 8 cores)."""
    key = ("nc", split_waits)
    if key in _prog_cache:
        return _prog_cache[key]

    _patch_tile_drain()
    import concourse.bass as bass
    import concourse.tile as tile
    from concourse import mybir
    from concourse.masks import make_identity

    FP = mybir.dt.float32
    BF = mybir.dt.bfloat16
    I16 = mybir.dt.int16

    nc = bass.Bass("TRN2", target_bir_lowering=False, debug=False)
    # NOTE: keep xt as strided [c, t] — multi-line descriptors get sprayed
    # across the 16 HW DMA queues in parallel; a contiguous 256KB single-line
    # descriptor lands on ONE queue (~12GB/s) and is ~3x slower end-to-end.
    xt = nc.dram_tensor("xt", [DIM_IMG, TK], BF, kind="ExternalInput").ap()
    xtt = nc.dram_tensor("xtt", [DIM_TXT, TQ], BF, kind="ExternalInput").ap()
    wk = nc.dram_tensor("wk", [DIM_IMG, HS], BF, kind="ExternalInput").ap()
    wq = nc.dram_tensor("wq", [DIM_TXT, HS], BF, kind="ExternalInput").ap()
    wv = nc.dram_tensor("wv", [DIM_IMG, HS], BF, kind="ExternalInput").ap()
    # folded rope tables [128, TK/2]: rows 0:64 even t-tiles, 64:128 odd;
    # col j = 128*P + t'  <->  t = 256*P + t' (+128 for the odd rows)
    cck = nc.dram_tensor("cck", [128, TK // 2], BF, kind="ExternalInput").ap()
    ssk = nc.dram_tensor("ssk", [128, TK // 2], BF, kind="ExternalInput").ap()
    ccq = nc.dram_tensor("ccq", [HS, TQ], BF, kind="ExternalInput").ap()
    ssq = nc.dram_tensor("ssq", [HS, TQ], BF, kind="ExternalInput").ap()
    out = nc.dram_tensor("out", [TQ, HS], FP, kind="ExternalOutput").ap()

    Exp = mybir.ActivationFunctionType.Exp
    Mul = mybir.AluOpType.mult
    Add = mybir.AluOpType.add
    NC4 = DIM_IMG // 128  # 4 c-chunks
    NT = TK // 128  # 32 t-tiles
    NP = NT // 2  # 16 scores pairs (tile 2P with tile 2P+1)
    A_DVE = float(SCALE * A16)

    with tile.TileContext(nc) as tc:
        with ExitStack() as ctx:
            const = ctx.enter_context(tc.tile_pool(name="const", bufs=1))
            pwp = ctx.enter_context(tc.tile_pool(name="pw", bufs=3, space="PSUM"))
            pop = ctx.enter_context(tc.tile_pool(name="po", bufs=2, space="PSUM"))
            esb = ctx.enter_context(tc.tile_pool(name="esb", bufs=4))
            osbp = ctx.enter_context(tc.tile_pool(name="osb", bufs=2))
            pkp = ctx.enter_context(tc.tile_pool(name="pkp", bufs=2))

            # ---- PE warm-up: the HAM clock gate keeps the PE at 1.2 GHz
            # until it sees ~3.4us of sustained activity. Run a train of tiny
            # matmuls during the DMA wait so real matmuls start at 2.4 GHz.
            wsrc = const.tile([1, 128], BF, tag="wsrc")
            nc.vector.memset(wsrc[:], 0.25)

            def warm_train(n, base):
                for w in range(n):
                    wt = pwp.tile([1, 128], FP, tag="psw", name=f"warm{base + w}")
                    nc.tensor.matmul(
                        wt[:], lhsT=wsrc[0:1, 0:1], rhs=wsrc[:], start=True, stop=True
                    )

            warm_train(48, 0)

            # ---- SBUF tiles ----
            wq_sb = const.tile([128, HS], BF, tag="wq")
            xtt_sb = const.tile([128, TQ], BF, tag="xtt")
            wk_sb = const.tile([128, NC4 * HS], BF, tag="wk")
            wv_sb = const.tile([128, NC4 * HS], BF, tag="wv")
            ccq_sb = const.tile([HS, TQ], BF, tag="ccq")
            ssq_sb = const.tile([HS, TQ], BF, tag="ssq")
            cck_sb = const.tile([128, TK // 2], BF, tag="cck")
            ssk_sb = const.tile([128, TK // 2], BF, tag="ssk")
            xt_sb = [const.tile([128, TK], BF, tag=f"xt{ci}", name=f"xt_sb{ci}")
                     for ci in range(NC4)]

            # ---- DMA: each dma_start occupies its host sequencer ~0.7us and
            # rings drain roughly serially; spread across FOUR rings with the
            # startup-critical wave first on each. xt transfers are contiguous
            # 256KB slab reads (dram layout [p][ci][128][1024]). ----
            def xt_dma(eng, p, ci):
                c0, c1 = 1024 * p, 1024 * (p + 1)
                eng.dma_start(xt_sb[ci][:, c0:c1],
                              xt[128 * ci:128 * (ci + 1), c0:c1])

            # scalar ring: q chunk-0 chain (Act consumes it first), then q1
            nc.scalar.dma_start(wq_sb[:], wq[:])
            nc.scalar.dma_start(xtt_sb[:, 0:512], xtt[:, 0:512])
            nc.scalar.dma_start(ccq_sb[:, 0:512], ccq[:, 0:512])
            nc.scalar.dma_start(ssq_sb[:, 0:512], ssq[:, 0:512])
            nc.scalar.dma_start(xtt_sb[:, 512:1024], xtt[:, 512:1024])
            nc.scalar.dma_start(ccq_sb[:, 512:1024], ccq[:, 512:1024])
            nc.scalar.dma_start(ssq_sb[:, 512:1024], ssq[:, 512:1024])
            # sync ring: k chain for slab 0, then bulk ci 0,1
            nc.sync.dma_start(
                wk_sb[:].rearrange("p (a h) -> p a h", a=NC4),
                wk.rearrange("(a p) h -> p a h", p=128),
            )
            nc.sync.dma_start(xt_sb[0][:, 0:512], xt[0:128, 0:512])
            nc.sync.dma_start(xt_sb[1][:, 0:512], xt[128:256, 0:512])
            nc.sync.dma_start(xt_sb[0][:, 512:1024], xt[0:128, 512:1024])
            nc.sync.dma_start(xt_sb[1][:, 512:1024], xt[128:256, 512:1024])
            nc.sync.dma_start(cck_sb[:, 0:512], cck[:, 0:512])
            for p in (1, 2, 3):
                xt_dma(nc.sync, p, 0)
                xt_dma(nc.sync, p, 1)
            nc.sync.dma_start(cck_sb[:, 512:2048], cck[:, 512:2048])
            # gpsimd ring: k chain tail + bulk ci 2,3
            nc.gpsimd.dma_start(xt_sb[2][:, 0:512], xt[256:384, 0:512])
            nc.gpsimd.dma_start(xt_sb[3][:, 0:512], xt[384:512, 0:512])
            nc.gpsimd.dma_start(xt_sb[2][:, 512:1024], xt[256:384, 512:1024])
            nc.gpsimd.dma_start(xt_sb[3][:, 512:1024], xt[384:512, 512:1024])
            nc.gpsimd.dma_start(ssk_sb[:, 0:512], ssk[:, 0:512])
            nc.gpsimd.dma_start(
                wv_sb[:].rearrange("p (a h) -> p a h", a=NC4),
                wv.rearrange("(a p) h -> p a h", p=128),
            )
            for p in (1, 2, 3):
                xt_dma(nc.gpsimd, p, 2)
                xt_dma(nc.gpsimd, p, 3)
            nc.gpsimd.dma_start(ssk_sb[:, 512:2048], ssk[:, 512:2048])
            # late q tables (needed from ~step 26)
            nc.gpsimd.dma_start(xtt_sb[:, 1024:2048], xtt[:, 1024:2048])
            nc.gpsimd.dma_start(ccq_sb[:, 1024:2048], ccq[:, 1024:2048])
            nc.gpsimd.dma_start(ssq_sb[:, 1024:2048], ssq[:, 1024:2048])

            ident = const.tile([128, 128], FP, tag="ident")
            make_identity(nc, ident[:])
            identb = const.tile([128, 128], BF, tag="identb")
            nc.gpsimd.tensor_copy(identb[:], ident[:])

            K2f = const.tile([128, TK // 2], BF, tag="K2f")
            Q2 = const.tile([128, TQ], BF, tag="Q2")
            qt_pre = const.tile([HS, TQ], BF, tag="qtpre")
            # v_aug: 32 t-tiles, 65 cols each (64 head dims + ones column)
            # (memset on vector: gpsimd's queue is busy issuing DMAs and the
            # ones column is needed by the first att at ~9us)
            # 80 cols per tile: 64 head dims + ones col + 15 zero pad so the
            # att output / osb have 80 rows (16-divisible for the DMA xbar)
            v_all = const.tile([128, NT * 80], BF, tag="vall")
            nc.vector.memset(v_all[:], 0.0)
            nc.vector.memset(v_all[:, HS::80], 1.0)

            # ---- q projection + rope ----
            def q_proj_chunk(j, cp):
                ps = pwp.tile([HS, 512], FP, tag="psw", name=f"psq{j}")
                nc.tensor.matmul(
                    ps[:], lhsT=wq_sb[:], rhs=xtt_sb[:, j * 512:(j + 1) * 512],
                    start=True, stop=True,
                )
                cp(qt_pre[:, j * 512:(j + 1) * 512], ps[:])

            def q_rope_chunk(j, on_gpsimd):
                cs = slice(j * 512, (j + 1) * 512)
                pq = pkp.tile([HS, 512], BF, tag="pq", name=f"pq{j}")
                nc.vector.stream_shuffle(pq[:], qt_pre[:, cs], SHUF_MASK)
                t1 = pkp.tile([HS, 512], BF, tag="t1q", name=f"t1q{j}")
                eng = nc.gpsimd if on_gpsimd else nc.vector
                eng.tensor_mul(t1[:], qt_pre[:, cs], ccq_sb[:, cs])
                eng.tensor_mul(pq[:], pq[:], ssq_sb[:, cs])
                eng.tensor_add(Q2[0:HS, cs], t1[:], pq[:])
                nc.vector.tensor_copy(Q2[HS:128, cs], Q2[0:HS, cs])

            # ---- k/v chunk-pair machinery ----
            # chunk-pair p = xt cols [1024p : 1024p+1024] = t-tiles 8p..8p+7;
            # K2f rows 0:64 <- first half of the slab (tiles 8p..8p+3), rows
            # 64:128 <- second half (tiles 8p+4..8p+7); scores pair P pairs
            # t-tiles (P + 4*(P//4), +4). Contiguous halves keep the k-matmul
            # rhs a plain column range (and only half the slab's DMA gates it).
            def k_mm(p, half, ps, cis):
                rb = half * HS
                c0 = 1024 * p + 512 * half
                for ci in cis:
                    nc.tensor.matmul(
                        ps[rb:rb + HS, :],
                        lhsT=wk_sb[:, ci * HS:(ci + 1) * HS],
                        rhs=xt_sb[ci][:, c0:c0 + 512],
                        start=(ci == 0), stop=(ci == NC4 - 1),
                    )

            def k_rope_dve(p, kps, c0=0, c1=512):
                # partner build + first mul on DVE (reads PSUM);
                # stream_shuffle requires same src/dst dtype -> fp32
                cs = slice(p * 512 + c0, p * 512 + c1)
                pk = pkp.tile([128, c1 - c0], FP, tag="pk", name=f"pk{p}_{c0}")
                nc.vector.stream_shuffle(pk[:], kps[:, c0:c1], SHUF_MASK)
                t1 = pkp.tile([128, c1 - c0], BF, tag="t1k", name=f"t1k{p}_{c0}")
                nc.vector.tensor_mul(t1[:], kps[:, c0:c1], cck_sb[:, cs])
                return pk, t1

            def k_rope_fin(p, pk, t1, on_gpsimd, c0=0, c1=512):
                cs = slice(p * 512 + c0, p * 512 + c1)
                eng = nc.gpsimd if on_gpsimd else nc.vector
                eng.tensor_mul(pk[:], pk[:], ssk_sb[:, cs])
                eng.tensor_add(K2f[:, cs], t1[:], pk[:])

            def v_mm(p, g, q, ps):
                # quad g covers tiles 8p+2g+{0,1,4,5} (the two att pairs
                # P=4p+2g, 4p+2g+1); psum col c=2q+j holds tile offset
                # [0,1,4,5][c] so att's (T, T+4) pairs complete together
                for j in range(2):
                    T = 8 * p + 2 * g + 4 * q + j
                    col = (2 * q + j) * HS
                    for ci in range(NC4):
                        nc.tensor.matmul(
                            ps[:, col:col + HS],
                            lhsT=xt_sb[ci][:, 128 * T:128 * (T + 1)],
                            rhs=wv_sb[:, ci * HS:(ci + 1) * HS],
                            start=(ci == 0), stop=(ci == NC4 - 1),
                        )

            def v_copy_quad(p, g, ps):
                # dst tiles T = 8p+2g+4j+q for (j, q) in 2x2
                dst = v_all[:].rearrange(
                    "part (S J G q hh) -> part S J G q hh", S=4, J=2, G=2, q=2, hh=80
                )[:, p, :, g, :, 0:HS]
                nc.vector.tensor_copy(
                    dst,
                    ps[:].rearrange("part (j q hh) -> part j q hh", j=2, q=2),
                )

            # ---- attention steps ----
            steps = [(0, 0), (0, 1), (1, 0), (1, 1)] + \
                    [(sc, P) for P in range(2, NP) for sc in (0, 1)] + \
                    [(2, P) for P in range(NP)] + [(3, P) for P in range(NP)]
            NSTEPS = len(steps)
            # DVE-exp steps: phase A i%4==2; phase B even i except the
            # epilogue steps (34, 50) where the DVE does recip/scale work
            DVE_STEPS = ({i for i in range(32) if i % 4 == 2} |
                         {i for i in range(32, 64) if i % 2 == 0} -
                         {34, 50}) | {39, 55}
            psos = {}

            def scores(i):
                sc, P = steps[i]
                psw = pwp.tile([128, 1024], FP, tag="psw", name=f"psw{sc}_{P}")
                for half in range(2):
                    rb = half * HS
                    nc.tensor.matmul(
                        psw[:, half * 512:(half + 1) * 512],
                        lhsT=K2f[rb:rb + HS, P * 128:(P + 1) * 128],
                        rhs=Q2[rb:rb + HS, sc * 512:(sc + 1) * 512],
                        start=True, stop=True,
                    )
                if i in DVE_STEPS:
                    eti = esb.tile([128, 1024], I16, tag="eti", name=f"eti{sc}_{P}")
                    return (psw, eti, True)
                et = esb.tile([128, 1024], BF, tag="et", name=f"et{sc}_{P}")
                return (psw, et, False)

            def expstep(i, pe):
                psw, et, dve = pe
                if dve:
                    nc.vector.tensor_scalar(et[:], psw[:], A_DVE, B16, Mul, Add)
                else:
                    nc.scalar.activation(et[:], psw[:], Exp, scale=SCALE)

            def att(i, pe):
                sc, P = steps[i]
                _, et, dve = pe
                eb = et[:].bitcast(BF) if dve else et[:]
                if sc not in psos:
                    psos[sc] = pop.tile([80, 512], FP, tag="pso", name=f"pso{sc}")
                T_lo = P + 4 * (P // 4)
                for half in range(2):
                    T = T_lo + 4 * half
                    nc.tensor.matmul(
                        psos[sc][:],
                        lhsT=v_all[:, T * 80:(T + 1) * 80],
                        rhs=eb.rearrange("p (g s) -> p g s", g=2)[:, half, :],
                        start=(P == 0 and half == 0), stop=(P == NP - 1 and half == 1),
                    )
                return (sc, P)

            def epilogue(sc):
                pso = psos.pop(sc)
                osb = osbp.tile([80, 512], BF, tag="osb", name=f"osb{sc}")
                nc.scalar.copy(osb[:], pso[:])  # Act has slack; DVE is hot
                out_sb = osbp.tile([128, 4 * HS], FP, tag="outsb", name=f"outsb{sc}")
                if sc < 3:
                    # transpose via the DMA xbar: frees ~1.1us of PE per
                    # epilogue and avoids churning the psw PSUM ring; the
                    # ~2.6us xbar latency hides under the next steps. The
                    # xbar needs 16-divisible partitions, so the Z row goes
                    # separately via a tiny strided (scatter) DMA.
                    ost = osbp.tile([128, 4 * 80], BF, tag="ost", name=f"ost{sc}")
                    nc.sync.dma_start_transpose(
                        ost[:].rearrange("p (b h) -> p b h", b=4), osb[:]
                    )
                    zr4 = osbp.tile([128, 4], FP, tag="zr", name=f"zr{sc}")
                    nc.vector.reciprocal(zr4[:], ost[:, HS::80])
                    for j in range(4):
                        nc.vector.tensor_scalar_mul(
                            out_sb[:, j * HS:(j + 1) * HS],
                            ost[:, j * 80:j * 80 + HS], zr4[:, j:j + 1]
                        )
                else:
                    # tail epilogue: PE is idle after the last att, and the
                    # PE-transpose chain is ~1.3us shorter than the xbar path
                    pst = pwp.tile([128, 4 * 66], BF, tag="psw", name=f"pst{sc}")
                    for j in range(4):
                        nc.tensor.transpose(
                            pst[:, j * 66:j * 66 + 65],
                            osb[0:65, j * 128:(j + 1) * 128], identb[0:65, 0:65]
                        )
                        zr = osbp.tile([128, 1], FP, tag="zr", name=f"zr{sc}_{j}")
                        nc.vector.reciprocal(
                            zr[:], pst[:, j * 66 + HS:j * 66 + HS + 1])
                        nc.vector.tensor_scalar_mul(
                            out_sb[:, j * HS:(j + 1) * HS],
                            pst[:, j * 66:j * 66 + HS], zr[:]
                        )
                # out DMA halves on two rings (issue in parallel)
                for half, eng in ((0, nc.sync), (1, nc.gpsimd)):
                    eng.dma_start(
                        out[sc * 512 + half * 256:sc * 512 + (half + 1) * 256, :]
                        .rearrange("(j p) h -> p j h", p=128),
                        out_sb[:, half * 128:(half + 1) * 128]
                        .rearrange("p (j h) -> p j h", j=2),
                    )

            # ---- prologue: q chunks 0,1 + chunk-pair 0 k ----
            q_proj_chunk(0, nc.scalar.copy)
            kps0 = pwp.tile([128, 512], FP, tag="psw", name="kps0")
            k_mm(0, 0, kps0, (0, 1))
            k_mm(0, 0, kps0, (2, 3))
            k_mm(0, 1, kps0, (0, 1))
            k_mm(0, 1, kps0, (2, 3))
            q_rope_chunk(0, on_gpsimd=False)
            # rope the first 128 cols first: scores(0) only needs K2f[:, 0:128],
            # so it can issue ~1.5us before the full chunk-pair rope finishes
            pk0a, t10a = k_rope_dve(0, kps0, 0, 128)
            k_rope_fin(0, pk0a, t10a, on_gpsimd=False, c0=0, c1=128)
            # chunk 1 rope on DVE too: gpsimd's queue is still draining its
            # DMA issues at this point and Q2 chunk 1 is needed by step 1
            q_proj_chunk(1, nc.scalar.copy)
            q_rope_chunk(1, on_gpsimd=False)
            pk0b, t10b = k_rope_dve(0, kps0, 128, 512)
            k_rope_fin(0, pk0b, t10b, on_gpsimd=False, c0=128, c1=512)
            # v-proj for slab 0 fills the PE idle while the DVE ropes K/Q
            for g in (0, 1):
                vp = pwp.tile([128, 256], FP, tag="psw", name=f"vps0{g}")
                v_mm(0, g, 0, vp)
                v_mm(0, g, 1, vp)
                v_copy_quad(0, g, vp)

            # extra-work schedule inside steps:
            #  i in 0..3: v-proj for chunk-pair 0 (needed by att from i=1)
            #  i in 0..23: build chunk-pair p=i//8+1 over its 8 steps
            #  i in 25..29: q chunks 2,3 (needed from i=32)
            #  epilogues at 33, 35, 49, end
            _cp = {}

            def extra(i):
                if i < 24:
                    # k matmuls one step later (ph1-2): slab p's DMA hasn't
                    # landed at ph0 and queueing them there stalls the PE
                    p, ph = i // 8 + 1, i % 8
                    if ph == 1:
                        _cp["kps"] = pwp.tile([128, 512], FP, tag="psw",
                                              name=f"kps{p}")
                        k_mm(p, 0, _cp["kps"], range(NC4))
                    elif ph == 2:
                        k_mm(p, 1, _cp["kps"], range(NC4))
                    elif ph == 3:
                        _cp["pk"], _cp["t1"] = k_rope_dve(p, _cp["kps"])
                    elif ph == 4:
                        k_rope_fin(p, _cp["pk"], _cp["t1"], on_gpsimd=True)
                        _cp["va"] = pwp.tile([128, 256], FP, tag="psw",
                                             name=f"vps{p}a")
                        v_mm(p, 0, 0, _cp["va"])
                    elif ph == 5:
                        v_mm(p, 0, 1, _cp["va"])
                        v_copy_quad(p, 0, _cp["va"])
                    elif ph == 6:
                        _cp["vb"] = pwp.tile([128, 256], FP, tag="psw",
                                             name=f"vps{p}b")
                        v_mm(p, 1, 0, _cp["vb"])
                    elif ph == 7:
                        v_mm(p, 1, 1, _cp["vb"])
                        v_copy_quad(p, 1, _cp["vb"])
                elif i == 25:
                    # odd steps run their exp on Act, leaving the DVE free
                    # for the q-projection copy + rope shuffle
                    q_proj_chunk(2, nc.vector.tensor_copy)
                    q_rope_chunk(2, on_gpsimd=True)
                elif i == 27:
                    q_proj_chunk(3, nc.vector.tensor_copy)
                    q_rope_chunk(3, on_gpsimd=True)
                elif i == 32:
                    epilogue(0)
                elif i == 33:
                    epilogue(1)
                elif i == 49:
                    epilogue(2)

            # ---- pipelined steps: scores one ahead of exp ----
            pe_cur = scores(0)
            pend_att = None
            for i in range(NSTEPS):
                pe_next = scores(i + 1) if i + 1 < NSTEPS else None
                expstep(i, pe_cur)
                extra(i)
                if pend_att is not None:
                    att(*pend_att)
                pend_att = (i, pe_cur)
                pe_cur = pe_next
            att(*pend_att)
            epilogue(3)

    if split_waits:
        _split_excess_waits(nc)
    _prog_cache[key] = nc
    return nc


def _perm_rows():
    """Row r (0..63) <- head component: quadrant q=r//32, j=r%32;
    j<16 -> even component 2*(16q+j), else odd partner 2*(16q+j-16)+1.
    Partner rows sit 16 apart within each 32-row quadrant (stream_shuffle)."""
    perm = np.empty(HS, dtype=np.int64)
    pair = np.empty(HS, dtype=np.int64)  # rope pair index of each row
    sgn = np.empty(HS, dtype=np.float32)  # -1 for a-rows (cos row gets -sin)
    for r in range(HS):
        q_, j = r // 32, r % 32
        p = 16 * q_ + (j % 16)
        is_b = j >= 16
        perm[r] = 2 * p + (1 if is_b else 0)
        pair[r] = p
        sgn[r] = 1.0 if is_b else -1.0
    return perm, pair, sgn


def make_in_maps(x_image, x_text_emb, freqs_latex, freqs_img_x, freqs_img_y, Wk, Wq, Wv):
    """Host-side prep: transpose/cast activations, permute+transpose weights,
    build rope cos/sin tables (k tables folded even/odd to [128, TK/2])."""
    perm, pair, sgn = _perm_rows()

    wk_dev = np.ascontiguousarray(np.asarray(Wk)[perm].T).astype(BF16)
    wq_dev = np.ascontiguousarray(np.asarray(Wq)[perm].T).astype(BF16)
    wv_dev = np.ascontiguousarray(np.asarray(Wv).T).astype(BF16)

    fx = np.asarray(freqs_img_x, dtype=np.float32)  # [TK, 16, 2]
    fy = np.asarray(freqs_img_y, dtype=np.float32)
    fl = np.asarray(freqs_latex, dtype=np.float32)  # [TQ, 32, 2]

    # K: rope pair p: p<16 -> fx[:, p], p>=16 -> fy[:, p-16]
    kf = np.concatenate([fx, fy], axis=1)  # [TK, 32, 2]
    cos_k = kf[:, pair, 0].T  # [64, TK]
    sin_k = kf[:, pair, 1].T * sgn[:, None]
    # fold slab halves: rows 0:64 <- t in [1024p, 1024p+512), rows 64:128 <-
    # t in [1024p+512, 1024p+1024), laid out slab-major (K2f col block P=4p+r)
    def fold(m):  # [64, TK] -> [128, TK/2]
        r = m.reshape(HS, 4, 2, 512)
        lo = r[:, :, 0, :].reshape(HS, TK // 2)
        hi = r[:, :, 1, :].reshape(HS, TK // 2)
        return np.concatenate([lo, hi], axis=0)
    cck = np.ascontiguousarray(fold(cos_k)).astype(BF16)
    ssk = np.ascontiguousarray(fold(sin_k)).astype(BF16)

    ccq = np.ascontiguousarray(fl[:, pair, 0].T).astype(BF16)  # [64, TQ]
    ssq = np.ascontiguousarray((fl[:, pair, 1].T * sgn[:, None])).astype(BF16)

    xi = np.asarray(x_image, dtype=np.float32)
    xte = np.asarray(x_text_emb, dtype=np.float32)
    in_maps = []
    for b in range(N_CORES):
        in_maps.append(
            {
                "xt": np.ascontiguousarray(xi[b].T).astype(BF16),
                "xtt": np.ascontiguousarray(xte[b].T).astype(BF16),
                "wk": wk_dev, "wq": wq_dev, "wv": wv_dev,
                "cck": cck, "ssk": ssk, "ccq": ccq, "ssq": ssq,
            }
        )
    return in_maps


def kernel(x_image, x_text_emb, x_latex_mask, freqs_latex, freqs_img_x, freqs_img_y,
           Wk, Wq, Wv):
    del x_latex_mask  # unused in the reference
    from concourse.bass_utils import run_bass_kernel_spmd

    nc = build_program()
    in_maps = make_in_maps(
        x_image, x_text_emb, freqs_latex, freqs_img_x, freqs_img_y, Wk, Wq, Wv
    )
    res = run_bass_kernel_spmd(nc, in_maps, list(range(N_CORES)))
    return np.stack([res.results[b]["out"] for b in range(N_CORES)], axis=0)
